# revision 3
# baseline (speedup 1.0000x reference)
"""Trainium2 Bass kernel for nn_Attention_83743272337693 — v6.

Same math as the baseline (q/k/v proj -> RoPE -> per-token-head int8 quant of
q,k -> exact int8 score GEMM -> causal softmax -> attn @ v -> o_proj; TP over
heads across 8 cores, host sums the Wo row-shard partials), restructured for
Tensor-engine continuity:

- f16 weights/activations for the projections; host pre-arranges all inputs
  into SBUF layouts so every DMA is contiguous per partition; weights stream
  on two HWDGE queues in arrival-ordered chunks (cold start ~5us).
- the per-row quant scale rq is folded into q right after quantization
  (qTs = q_int * rq, f32r), so attention scores need no per-row scaling.
- scores are computed TRANSPOSED: S^T[k, (h,q)] = kTs_blk^T @ qTs — the
  moving operand packs all 4 heads (512 wide, full PE rate).  exp() then
  writes straight from PSUM into the [k, h, q] layout attn@v needs: no
  PE/DMA transposes of P, no SBUF staging of scores or probabilities.
- softmax drops the max-subtraction (logits are O(10); exp'd into bf16).
  Row sums come from a ones-row matmul over the unnormalized P^T;
  normalization is one multiply on the attn-output columns.
- o_proj is pipelined two q-tiles behind and its matmuls are interleaved
  into the S^T stream, hiding the softmax/normalize chains entirely.
- tile 15's transposes + the tail of the rk broadcast are emitted inside
  phase B (shared PSUM pool), so the A->B transition costs nothing.
"""
import numpy as np

import concourse.bass as bass
import concourse.mybir as mybir
from concourse import bacc, bass_utils
from concourse.tile import TileContext
from concourse.masks import make_causal_mask, make_identity

B, S, D = 1, 2048, 4096
NH, NKV, HD = 32, 8, 128
N_CORES = 8
HQ = NH // N_CORES          # query heads per core (4)
ST = S // 128               # seq tiles (16)
KC = D // 128               # contraction chunks for projections (32)
SCALE = float(HD) ** -0.5
MAGIC = float(np.float32(1.5 * 2 ** 23))
MASK_VAL = -1.0e10

F32 = mybir.dt.float32
F32R = mybir.dt.float32r
BF16 = mybir.dt.bfloat16
F16 = mybir.dt.float16


def build(debug=False):
    nc = bacc.Bacc("TRN2", target_bir_lowering=False)

    # host-prearranged layouts (see make_in_maps)
    x3 = nc.dram_tensor("x3", [128, ST, KC, 128], F16, kind="ExternalInput")
    cos3 = nc.dram_tensor("cos3", [128, ST, HD // 2], F32, kind="ExternalInput")
    sin3 = nc.dram_tensor("sin3", [128, ST, HD // 2], F32, kind="ExternalInput")
    wq3 = nc.dram_tensor("wq3", [128, KC, HQ * HD], F16, kind="ExternalInput")
    wkv3 = nc.dram_tensor("wkv3", [128, KC, 2 * HD], F16, kind="ExternalInput")
    wo3 = nc.dram_tensor("wo3", [128, HQ, D], F16, kind="ExternalInput")
    y = nc.dram_tensor("y", [S, D], F16, kind="ExternalOutput")

    with TileContext(nc) as tc:
        with (
            tc.tile_pool(name="persist", bufs=1) as persist,
            tc.tile_pool(name="small", bufs=4) as small,
            tc.tile_pool(name="aux1", bufs=1) as aux1,
            tc.tile_pool(name="ropebuf", bufs=2) as rpool,
            tc.tile_pool(name="xstream", bufs=2) as xpool,
            # psS serves the phase-A transposes AND phase-B S^T chunks (3
            # banks, whole kernel) so tile-15's transposes can be emitted
            # inside phase B, erasing the A->B transition bubble.
            tc.tile_pool(name="psS", bufs=3, space="PSUM") as psS,
        ):
            qTs = persist.tile([128, HQ, S], F32R, tag="qTs")      # 4 MiB
            kTs = persist.tile([128, S], F32R, tag="kTs")          # 1 MiB
            v_sb = persist.tile([128, ST, HD], BF16, tag="v_sb")   # 512 KiB
            rkcols = persist.tile([128, ST], F32, tag="rkcols")
            ident_bf = persist.tile([128, 128], BF16, tag="ident_bf")
            ident_f32 = persist.tile([128, 128], F32, tag="ident_f32")
            ident_fr = persist.tile([128, 128], F32R, tag="ident_fr")
            mask_lo = persist.tile([128, 128], F32, tag="mask_lo")
            ones_f = persist.tile([1, 128], F32, tag="ones_f")
            ones_sb = persist.tile([1, 128], F32R, tag="ones_sb")
            onec_f = persist.tile([128, 1], F32, tag="onec_f")
            onec_sb = persist.tile([128, 1], BF16, tag="onec_sb")
            wo_sb = persist.tile([128, HQ, D], F16, tag="wo_sb")    # 4 MiB
            cos_sb = persist.tile([128, ST, HD // 2], F32, tag="cos_sb")
            sin_sb = persist.tile([128, ST, HD // 2], F32, tag="sin_sb")

            # prefetch the first two x tiles as the very first gpsimd-queue
            # instructions — the memset/identity preamble otherwise delays
            # the first projection by ~5us
            xt_pre0 = xpool.tile([128, KC, 128], F16, tag="xt")
            xt_pre1 = xpool.tile([128, KC, 128], F16, tag="xt")
            xt_pre = [xt_pre0, xt_pre1]
            nc.gpsimd.dma_start(xt_pre0[:], x3.ap()[:, 0, :, :])
            nc.gpsimd.dma_start(xt_pre1[:], x3.ap()[:, 1, :, :])

            make_identity(nc, ident_bf[:])
            make_identity(nc, ident_f32[:])
            nc.vector.tensor_copy(ident_fr[:], ident_f32[:])
            nc.gpsimd.memset(ones_f[:], 1.0)
            nc.vector.tensor_copy(ones_sb[:], ones_f[:])
            nc.gpsimd.memset(onec_f[:], 1.0)
            nc.vector.tensor_copy(onec_sb[:], onec_f[:])

            def emit_transposes(st, qs_f, kint):
                # q heads: f32r transposes (values are q_int*rq);
                # k head: bf16 int transpose.
                ps_tq = psS.tile([128, 512], F32R, tag="ps_ST")
                for hh in range(HQ):
                    nc.tensor.transpose(ps_tq[:, hh * 128:(hh + 1) * 128],
                                        qs_f[:, hh, :], ident_fr[:])
                ps_tk = psS.tile([128, 128], F32, tag="ps_ST")
                nc.tensor.matmul(ps_tk[:], kint[:, 0, :], ident_bf[:])
                nc.scalar.copy(
                    qTs[:, :, st * 128:(st + 1) * 128],
                    ps_tq[:].rearrange("p (h q) -> p h q", h=4))
                nc.scalar.copy(kTs[:, st * 128:(st + 1) * 128], ps_tk[:])

            rk_row = aux1.tile([1, ST, 128], F32R, tag="rk_row")
            rk_flat = rk_row[:].rearrange("o t s -> o (t s)")
            kf = kTs[:].bitcast(F32)

            # ---------------- Phase A: projections + rope + quantize ----------
            with (
                tc.tile_pool(name="wproj", bufs=1) as wpool,
                tc.tile_pool(name="psA", bufs=2, space="PSUM") as psA,
            ):
                # cos/sin + wkv on the scalar queue, wq + wo on sync: two
                # parallel HWDGE streams so tile-0 never waits on weights.
                nc.scalar.dma_start(cos_sb[:], cos3.ap())
                nc.scalar.dma_start(sin_sb[:], sin3.ap())
                # upper-tri causal mask, then transpose -> lower-tri (for S^T)
                mask_hi = aux1.tile([128, 128], F32, tag="mask_hi")
                make_causal_mask(nc, mask_hi[:], mask_val=MASK_VAL)
                ps_m = psS.tile([128, 128], F32, tag="ps_ST")
                nc.tensor.transpose(ps_m[:], mask_hi[:], ident_f32[:])
                nc.vector.tensor_copy(mask_lo[:], ps_m[:])

                wq_sb = wpool.tile([128, KC, HQ * HD], F16, tag="wq_sb")
                wkv_sb = wpool.tile([128, KC, 2 * HD], F16, tag="wkv_sb")
                kc0 = 0
                for step in (2, 2, 2, 2, 4, 4, 4, 4, 4, 4):
                    nc.sync.dma_start(wq_sb[:, kc0:kc0 + step, :],
                                      wq3.ap()[:, kc0:kc0 + step, :])
                    nc.scalar.dma_start(wkv_sb[:, kc0:kc0 + step, :],
                                        wkv3.ap()[:, kc0:kc0 + step, :])
                    kc0 += step
                # wo arrives during phase A; oproj needs it ~8us into phase B
                nc.sync.dma_start(wo_sb[:], wo3.ap())

                prev = None
                for st in range(ST):
                    if st < 2:
                        xt = xt_pre[st]
                    else:
                        xt = xpool.tile([128, KC, 128], F16, tag="xt")
                        nc.gpsimd.dma_start(xt[:], x3.ap()[:, st, :, :])

                    ps_a = psA.tile([128, (HQ + 2) * HD], F32, tag="ps_a")
                    for kc in range(KC):
                        nc.tensor.matmul(ps_a[:, 0:512], xt[:, kc, :],
                                         wq_sb[:, kc, :],
                                         start=(kc == 0), stop=(kc == KC - 1))
                        nc.tensor.matmul(ps_a[:, 512:768], xt[:, kc, :],
                                         wkv_sb[:, kc, :],
                                         start=(kc == 0), stop=(kc == KC - 1))
                        if st == ST - 1 and kc == 8:
                            # rk row build for tiles 0..14, emitted mid-proj:
                            # the transpose's input (quant(14)) is ready and
                            # the DVE copy + row DMA complete long before the
                            # ones-broadcast matmuls below need them.
                            ps_rt = psS.tile([16, 128], F32, tag="ps_ST")
                            nc.tensor.transpose(ps_rt[0:15, :],
                                                rkcols[:, 0:15], ident_f32[:])
                            rk_rowT = aux1.tile([16, 128], F32R, tag="rk_rowT")
                            nc.vector.tensor_copy(rk_rowT[0:15, :],
                                                  ps_rt[0:15, :])
                            nc.sync.dma_start(rk_row[:, 0:15, :],
                                              rk_rowT[0:15, :])
                    # transposes of the PREVIOUS tile: their DVE inputs are
                    # ready, so the PE never waits on the rope/quant chain.
                    if prev is not None:
                        emit_transposes(st - 1, *prev)

                    if st == ST - 1:
                        # rk broadcast part 1 (tiles 0..14): ones-matmul per
                        # 512-chunk, kTs multiplied straight from PSUM (no
                        # SBUF staging), chunk-interleaved so S^T(qt=0)
                        # unblocks right after the first multiply.
                        b0 = 0
                        for w in (512, 512, 512, 384):
                            ps_b = psS.tile([128, 512], F32, tag="ps_ST")
                            nc.tensor.matmul(ps_b[:, :w], ones_sb[:],
                                             rk_flat[:, b0:b0 + w])
                            nc.vector.tensor_tensor(
                                kTs[:, b0:b0 + w], kf[:, b0:b0 + w],
                                ps_b[:, :w], op=mybir.AluOpType.mult)
                            b0 += w

                    # RoPE for the 4 q heads + 1 k head, fused via [128,5,64]
                    # strided views of PSUM + broadcast cos/sin.
                    half = HD // 2
                    heads = ps_a[:, 0:5 * HD].rearrange("p (h d) -> p h d", h=5)
                    x1 = heads[:, :, 0:half]
                    x2 = heads[:, :, half:HD]
                    cos_b = cos_sb[:, st, :].unsqueeze(1).broadcast_to([128, 5, half])
                    sin_b = sin_sb[:, st, :].unsqueeze(1).broadcast_to([128, 5, half])
                    rope = rpool.tile([128, 5, HD], F32, tag="rope")
                    tmp = rpool.tile([128, 5, half], F32, tag="tmp")
                    r1 = rope[:, :, 0:half]
                    r2 = rope[:, :, half:HD]
                    nc.vector.tensor_tensor(r1, x1, cos_b, op=mybir.AluOpType.mult)
                    nc.vector.tensor_tensor(tmp[:], x2, sin_b, op=mybir.AluOpType.mult)
                    nc.vector.tensor_tensor(r1, r1, tmp[:], op=mybir.AluOpType.subtract)
                    nc.vector.tensor_tensor(r2, x1, sin_b, op=mybir.AluOpType.mult)
                    nc.vector.tensor_tensor(tmp[:], x2, cos_b, op=mybir.AluOpType.mult)
                    nc.vector.tensor_tensor(r2, r2, tmp[:], op=mybir.AluOpType.add)

                    # v: straight cast to bf16
                    nc.scalar.copy(v_sb[:, st, :], ps_a[:, 640:768])

                    # quantize all 5 heads at once; then fold rq into q
                    am = small.tile([128, 5], F32, tag="am")
                    nc.vector.tensor_reduce(am[:], rope[:], axis=mybir.AxisListType.X,
                                            op=mybir.AluOpType.max,
                                            apply_absolute_value=True)
                    nc.vector.tensor_scalar_max(am[:], am[:], 1e-5)
                    nc.vector.tensor_scalar_mul(rkcols[:, st:st + 1],
                                                am[:, 4:5], 1.0 / 127.0)
                    sc = small.tile([128, 5], F32, tag="sc")
                    nc.vector.reciprocal(sc[:], am[:])
                    nc.vector.tensor_scalar_mul(sc[:], sc[:], 127.0)
                    rnd = rpool.tile([128, 5, HD], F32, tag="rnd")
                    sc_b = sc[:].unsqueeze(2).broadcast_to([128, 5, HD])
                    nc.vector.tensor_tensor(rnd[:], rope[:], sc_b,
                                            op=mybir.AluOpType.mult)
                    qint = rpool.tile([128, 5, HD], F32, tag="qint")
                    nc.vector.tensor_scalar(qint[:], rnd[:], MAGIC, -MAGIC,
                                            op0=mybir.AluOpType.add,
                                            op1=mybir.AluOpType.add)
                    # rq = am * SCALE/127 per (row, head); qs = q_int * rq
                    rq = small.tile([128, 5], F32, tag="rq")
                    nc.vector.tensor_scalar_mul(rq[:], am[:], SCALE / 127.0)
                    qs_f = rpool.tile([128, HQ, HD], F32R, tag="qs_f")
                    rq_b = rq[:, 0:4].unsqueeze(2).broadcast_to([128, 4, HD])
                    nc.vector.tensor_tensor(qs_f[:], qint[:, 0:4, :], rq_b,
                                            op=mybir.AluOpType.mult)
                    kint = rpool.tile([128, 1, HD], BF16, tag="kint")
                    nc.vector.tensor_copy(kint[:], qint[:, 4:5, :])
                    prev = (qs_f, kint)

            # ---------------- Phase B: attention + o_proj ---------------------
            with (
                tc.tile_pool(name="pbuf", bufs=2) as pbuf,
                tc.tile_pool(name="ohbuf", bufs=3) as ohbuf,
                tc.tile_pool(name="obuf", bufs=3) as obuf,
                tc.tile_pool(name="psSumW", bufs=1, space="PSUM") as psSumW,
                tc.tile_pool(name="psV", bufs=1, space="PSUM") as psV,
                tc.tile_pool(name="psO", bufs=3, space="PSUM") as psO,
            ):
                if debug:
                    nc.gpsimd.dma_start(
                        y.ap()[0:128, 0:512].rearrange("p (h q) -> p h q", h=4),
                        qTs[:, :, 0:128])
                    nc.gpsimd.dma_start(y.ap()[0:128, 512:640], kTs[:, 0:128])
                    nc.gpsimd.dma_start(y.ap()[0:128, 704:720], rkcols[:])

                def oproj_mms(j, ohT_j):
                    """Generator of the 32 o_proj matmuls + per-group drains
                    for q-tile j; consumed interleaved with S^T matmuls."""
                    for g in range(8):
                        ps_O = psO.tile([128, 512], F32, tag="ps_O")
                        for h in range(HQ):
                            w0 = g * 512
                            yield ("mm", ps_O, h, w0, ohT_j)
                        yield ("out", ps_O, g, j)

                def emit_oproj_step(step):
                    kind = step[0]
                    if kind == "mm":
                        _, ps_O, h, w0, ohT_j = step
                        nc.tensor.matmul(ps_O[:],
                                         ohT_j[:, h * 128:(h + 1) * 128],
                                         wo_sb[:, h, w0:w0 + 512],
                                         start=(h == 0), stop=(h == HQ - 1))
                    else:
                        _, ps_O, g, j = step
                        out_t = obuf.tile([128, 512], F16, tag="out_t")
                        nc.vector.tensor_copy(out_t[:], ps_O[:])
                        nc.gpsimd.dma_start(
                            y.ap()[j * 128:(j + 1) * 128,
                                   g * 512:(g + 1) * 512], out_t[:])

                ohT_hist = {}
                pending = None  # iterator of oproj steps for q-tile qt-2

                for qt in range(ST if not debug else 2):
                    pTt = pbuf.tile([128, ST, HQ, 128], BF16, tag="pTt")
                    rhs_q = qTs[:, :, qt * 128:(qt + 1) * 128]
                    if pending is None and qt >= 2 and not debug:
                        pending = oproj_mms(qt - 2, ohT_hist.pop(qt - 2))
                    for kc in range(qt + 1):
                        ps_ST = psS.tile([128, HQ * 128], F32, tag="ps_ST")
                        nc.tensor.matmul(ps_ST[:],
                                         kTs[:, kc * 128:(kc + 1) * 128],
                                         rhs_q)
                        # interleave 2 o_proj steps per S^T block: the PE
                        # never stalls on exp() draining PSUM slots, and a
                        # few steps remain to cover the sums/attnV lead-in
                        if pending is not None:
                            for _ in range(2):
                                step = next(pending, None)
                                if step is None:
                                    pending = None
                                    break
                                emit_oproj_step(step)
                        if kc == qt:  # diagonal block: lower-tri mask per head
                            mview = ps_ST[:].rearrange("p (h q) -> p h q", h=4)
                            mb = mask_lo[:].unsqueeze(1).broadcast_to(
                                [128, 4, 128])
                            nc.vector.tensor_tensor(mview, mview, mb,
                                                    op=mybir.AluOpType.add)
                        nc.scalar.activation(
                            pTt[:, kc, :, :], ps_ST[:],
                            mybir.ActivationFunctionType.Exp)

                    if debug and qt == 1:
                        nc.gpsimd.dma_start(
                            y.ap()[256:384, 0:1024]
                            .rearrange("p (b hq) -> p b hq", b=2),
                            pTt[:, 0:2, :, :]
                            .rearrange("p b h q -> p b (h q)"))

                    # row sums of unnormalized P^T via ones-row matmul
                    ps_sum = psSumW.tile([1, HQ * 128], F32, tag="ps_sw")
                    for kc in range(qt + 1):
                        nc.tensor.matmul(ps_sum[:], onec_sb[:],
                                         pTt[:, kc, :, :],
                                         start=(kc == 0), stop=(kc == qt))
                        if pending is not None:
                            step = next(pending, None)
                            if step is None:
                                pending = None
                            else:
                                emit_oproj_step(step)
                    ps_oh = psV.tile([128, HQ * 128], F32, tag="ps_oh")
                    for kc in range(qt + 1):
                        nc.tensor.matmul(
                            ps_oh[:], v_sb[:, kc, :],
                            pTt[:, kc, :, :],
                            start=(kc == 0), stop=(kc == qt))
                        if pending is not None:
                            step = next(pending, None)
                            if step is None:
                                pending = None
                            else:
                                emit_oproj_step(step)
                    # drain whatever oproj work the interleaves didn't cover,
                    # so every q-tile's oproj completes within its window
                    if pending is not None:
                        for step in pending:
                            emit_oproj_step(step)
                        pending = None
                    tail14 = None
                    if qt == ST - 1 and not debug:
                        # oproj(14): first half before ps_w(15) so the s_row
                        # copy hides; second half after, covering the recip +
                        # normalize chain that gates oproj(15)
                        tail14 = oproj_mms(ST - 2, ohT_hist.pop(ST - 2))
                        for _ in range(20):
                            emit_oproj_step(next(tail14))

                    s_row = small.tile([1, HQ * 128], F32R, tag="s_row")
                    nc.vector.tensor_copy(s_row[:], ps_sum[:])
                    ps_w = psSumW.tile([128, HQ * 128], F32, tag="ps_sw")
                    nc.tensor.matmul(ps_w[:], ones_sb[:], s_row[:])
                    if tail14 is not None:
                        for step in tail14:
                            emit_oproj_step(step)
                    w_bc = pbuf.tile([128, HQ * 128], F32, tag="w_bc")
                    nc.vector.reciprocal(w_bc[:], ps_w[:])
                    ohT = ohbuf.tile([128, HQ * 128], F16, tag="ohT")
                    nc.vector.tensor_tensor(ohT[:], ps_oh[:], w_bc[:],
                                            op=mybir.AluOpType.mult)
                    if debug and qt == 1:
                        nc.gpsimd.dma_start(y.ap()[384:512, 0:512], ohT[:])
                        nc.gpsimd.dma_start(y.ap()[384:512, 512:1024], w_bc[:])
                    ohT_hist[qt] = ohT

                    if qt == 1:
                        # tile 15's transposes + rk tail, tucked between the
                        # early (tiny) q-tiles: by now tile 15's rope/quant
                        # has long drained; nothing here is on a critical
                        # path until qt==15.
                        emit_transposes(ST - 1, *prev)
                        ps_rt2 = psS.tile([16, 128], F32, tag="ps_ST")
                        nc.tensor.transpose(ps_rt2[0:1, :], rkcols[:, 15:16],
                                            ident_f32[:])
                        rk_rowT2 = aux1.tile([1, 128], F32R, tag="rk_rowT2")
                        nc.vector.tensor_copy(rk_rowT2[:], ps_rt2[0:1, :])
                        nc.sync.dma_start(rk_row[:, 15:16, :], rk_rowT2[:])
                        ps_b2 = psS.tile([128, 512], F32, tag="ps_ST")
                        nc.tensor.matmul(ps_b2[:, 0:128], ones_sb[:],
                                         rk_flat[:, 1920:2048])
                        nc.vector.tensor_tensor(kTs[:, 1920:2048],
                                                kf[:, 1920:2048],
                                                ps_b2[:, 0:128],
                                                op=mybir.AluOpType.mult)

                if not debug:
                    for step in oproj_mms(ST - 1, ohT_hist.pop(ST - 1)):
                        emit_oproj_step(step)

    nc.finalize()
    return nc


_NC_CACHE = None


def _get_nc():
    global _NC_CACHE
    if _NC_CACHE is None:
        _NC_CACHE = build()
    return _NC_CACHE


def make_in_maps(x, cos, sin, Wq, Wk, Wv, Wo):
    """Shard + pre-arrange the full inputs into the 8 per-core input maps."""
    x = np.asarray(x, np.float32).reshape(S, D)
    # x3[dp, st, kc, sq] = x[st*128+sq, kc*128+dp]
    x3 = np.ascontiguousarray(
        x.reshape(ST, 128, KC, 128).transpose(3, 0, 2, 1)).astype(np.float16)
    cos = np.asarray(cos, np.float32)
    sin = np.asarray(sin, np.float32)
    # cos3[sq, st, :] = cos[st*128+sq, :]
    cos3 = np.ascontiguousarray(cos.reshape(ST, 128, HD // 2).transpose(1, 0, 2))
    sin3 = np.ascontiguousarray(sin.reshape(ST, 128, HD // 2).transpose(1, 0, 2))
    Wq = np.asarray(Wq, np.float32)
    Wk = np.asarray(Wk, np.float32)
    Wv = np.asarray(Wv, np.float32)
    Wo = np.asarray(Wo, np.float32)
    in_maps = []
    for c in range(N_CORES):
        qs = slice(c * HQ * HD, (c + 1) * HQ * HD)
        ks = slice(c * HD, (c + 1) * HD)
        wq3 = np.ascontiguousarray(
            Wq[:, qs].reshape(KC, 128, HQ * HD).transpose(1, 0, 2)
        ).astype(np.float16)
        wkv = np.concatenate([Wk[:, ks], Wv[:, ks]], axis=1)
        wkv3 = np.ascontiguousarray(
            wkv.reshape(KC, 128, 2 * HD).transpose(1, 0, 2)).astype(np.float16)
        wo3 = np.ascontiguousarray(
            Wo[qs, :].reshape(HQ, 128, D).transpose(1, 0, 2)).astype(np.float16)
        in_maps.append({
            "x3": x3, "cos3": cos3, "sin3": sin3,
            "wq3": wq3, "wkv3": wkv3, "wo3": wo3,
        })
    return in_maps


def run(x, cos, sin, Wq, Wk, Wv, Wo, trace=False):
    nc = _get_nc()
    in_maps = make_in_maps(x, cos, sin, Wq, Wk, Wv, Wo)
    res = bass_utils.run_bass_kernel_spmd(
        nc, in_maps, core_ids=list(range(N_CORES)), trace=trace)
    partials = np.stack([res.results[c]["y"].astype(np.float32)
                         for c in range(N_CORES)])
    out = partials.sum(axis=0)
    return out.reshape(B, S, D), res


def kernel(x, cos, sin, Wq, Wk, Wv, Wo):
    out, _ = run(x, cos, sin, Wq, Wk, Wv, Wo, trace=False)
    return out
